# revision 49
# baseline (speedup 1.0000x reference)
"""Trainium2 Bass kernel for expected-calibration-error (ECE) over [N,C] logits.

Contract: kernel(logits, targets) -> np.float32 scalar (shape ()), matching

    probs = softmax(logits); conf = max(probs); pred = argmax(probs)
    acc = (pred == targets); bins of width 1/10 over (k/10, (k+1)/10]
    ECE = sum_k |avg_conf_k - avg_acc_k| * count_k / N

Strategy (data-parallel over 8 NeuronCores, rows sharded):
  * Host converts logits to fp16 and gathers the target-class logit per
    row (tl). This halves HBM traffic and unlocks the DVE 2x perf mode
    (2-byte dtypes) for tensor_tensor fold chains. ECE is an aggregate
    over 262k rows with a 2e-2 tolerance; fp16's 1e-3-level noise on conf
    and ~0.2%-of-rows argmax-tie noise on acc are far below the gate.
  * Per tile [128 partitions, T rows, 128 classes], both row reductions
    run as DVE tensor_tensor fold chains 128->64->32->16 at the fp16 2x
    perf mode, then a short 1x tensor_reduce. GpSimd stays off the
    shared DVE/GpSimd SBUF port, so the TTs hold their 2x rate (a
    concurrent GpSimd fold was measured to halve DVE TT throughput).
  * acc = (tl == rowmax), exact in the fp16 domain (tl is one of the
    row's values, so equality holds iff the target attains the max).
  * conf = exp(rowmax) * reciprocal_approx(sumexp).
  * Bin masks as cumulative +-1 masks sign(conf - k/10), engine-split to
    balance load: groups 0-2 on the scalar engine (11 per-bin Sign
    activations with the negated threshold as per-partition bias; Exp
    and Sign share one act table set), the tail group on the DVE
    (is_gt broadcast, then a single-src 2x fixup g -> 2g-1 so one PSUM
    convention holds). The tensor engine accumulates cumulative [3,11]
    (count,conf,acc) sign-stats in PSUM via one tiny fp16 matmul per
    128-row block; the host recovers G = (G' + G'[:,0:1])/2, sums the
    cores' [3,11], differences adjacent columns, applies ECE.
  * The first tiles are small (head ramp): compute starts as soon as the
    first 0.5 MB lands instead of after a full 2 MB tile.

Dormant knobs kept for reference: USE_CCE_FOLD=True folds the sumexp via
in-place CCE-add DMAs. Measured verdict: works only when each accum DMA
stays <= ~4096 descriptors (32 rows x 128 partitions; bigger wedges the
SWDGE descriptor ring at runtime), and is uneconomical regardless — Q7
descriptor generation occupies gpsimd 2-3.4us per strided fold DMA,
matching the DVE TT it replaces while serializing the pipeline (126us
vs 66us measured). masks_eng='gpsimd' (gpsimd-subtract + scalar-engine
Sign masks) is untested in isolation; finalize(sign_masks=True) holds
the matching host-side math.
"""

import numpy as np

# Problem constants (hardcoded per harness contract).
N = 262144
C = 128
P = 128          # SBUF partitions
NB = 10          # calibration bins
NCORES = 8
ROWS_PER_CORE = N // NCORES          # 32768
TW = ROWS_PER_CORE // P              # 256 row-blocks (columns) per core

# tile segments: (rows-per-partition, count). Must sum to TW.
SEGMENTS = ((16, 2), (32, 1), (64, 3))
GW = 64          # phase-2 group width (columns)

_CACHE = {}

USE_CCE_FOLD = False

# masks_eng: 'gpsimd' = gpsimd subtract + ACT sign; 'dve' = DVE is_gt
KNOBS = dict(masks_eng="dve")


def build(segments=SEGMENTS, gw=GW, masks_eng="gpsimd"):
    """Build the Bass module. Returns nc."""
    import concourse.bacc as bacc
    import concourse.tile as tile
    from concourse import mybir

    f32 = mybir.dt.float32
    f16 = mybir.dt.float16
    Alu = mybir.AluOpType
    Act = mybir.ActivationFunctionType
    X = mybir.AxisListType.X

    tiles = []  # (col_offset, t_rows)
    off = 0
    for t_rows, cnt in segments:
        for _ in range(cnt):
            tiles.append((off, t_rows))
            off += t_rows
    assert off == TW
    assert TW % gw == 0
    ngroups = TW // gw

    nc = bacc.Bacc(trn_type="TRN2")

    y_d = nc.dram_tensor("y", [P, TW * C], f16, kind="ExternalInput")
    tl_d = nc.dram_tensor("tl", [P, TW], f16, kind="ExternalInput")
    thr_d = nc.dram_tensor("thr", [1, NB + 1], f16, kind="ExternalInput")
    out_d = nc.dram_tensor("gstats", [3, NB + 1], f32, kind="ExternalOutput")

    with tile.TileContext(nc) as tc:
        with (
            tc.tile_pool(name="io", bufs=6) as io_pool,
            tc.tile_pool(name="ep", bufs=3) as e_pool,
            tc.tile_pool(name="fp", bufs=3) as f_pool,
            tc.tile_pool(name="grp", bufs=3) as grp_pool,
            tc.tile_pool(name="single", bufs=1) as single,
            tc.tile_pool(name="psum", bufs=1, space="PSUM") as psum_pool,
        ):
            # Pre-issue the first y-tile DMAs (head ramp).
            y_tiles = {}
            for t in range(len(tiles)):
                o, tr = tiles[t]
                y_t = io_pool.tile([P, 64 * C], f16, name="y_t")
                nc.sync.dma_start(
                    out=y_t[:, : tr * C], in_=y_d[:, o * C : (o + tr) * C]
                )
                y_tiles[t] = y_t

            tl_all = single.tile([P, TW], f16)
            nc.sync.dma_start(out=tl_all[:], in_=tl_d[:])
            thr_sb = single.tile([P, NB + 1], f16)
            nc.sync.dma_start(out=thr_sb[:], in_=thr_d[:].partition_broadcast(P))
            # negated thresholds (fp32) as per-partition bias for ACT Sign
            thr_neg = single.tile([P, NB + 1], f32)
            nc.vector.tensor_scalar(
                out=thr_neg[:], in0=thr_sb[:], scalar1=-1.0, scalar2=None,
                op0=Alu.mult,
            )

            pstats = psum_pool.tile([3, NB + 1], f32)
            my_all = single.tile([P, TW], f16)
            s_all = single.tile([P, TW], f32)
            rhs3 = single.tile([P, 3, TW], f16)
            nc.gpsimd.memset(rhs3[:, 0, :], 1.0)

            def max_chain(src3, t_rows, out_col):
                """DVE TT max fold chain 128->64->32->16, then reduce."""
                cur, w = src3, C
                while w > 16:
                    h = w // 2
                    M = f_pool.tile([P, 64 * 64], f16, name="Mv")
                    M3 = M[:, : t_rows * h].rearrange("p (t c) -> p t c", c=h)
                    nc.vector.tensor_tensor(
                        out=M3, in0=cur[:, :, 0:h], in1=cur[:, :, h:w], op=Alu.max
                    )
                    cur, w = M3, h
                nc.vector.tensor_reduce(out=out_col, in_=cur, axis=X, op=Alu.max)

            def phase1(t):
                o, tr = tiles[t]
                y_t = y_tiles.pop(t, None)
                if y_t is None:
                    y_t = io_pool.tile([P, 64 * C], f16, name="y_t")
                    nc.sync.dma_start(
                        out=y_t[:, : tr * C], in_=y_d[:, o * C : (o + tr) * C]
                    )
                y3 = y_t[:, : tr * C].rearrange("p (t c) -> p t c", c=C)

                # row max: DVE fold chain (2x TT) + short reduce
                max_chain(y3, tr, my_all[:, o : o + tr])

                # row sumexp: fp16 exp, then a DVE add fold chain + reduce
                E = e_pool.tile([P, 64 * C], f16, name="E")
                nc.scalar.activation(out=E[:, : tr * C], in_=y_t[:, : tr * C],
                                     func=Act.Exp)
                E3 = E[:, : tr * C].rearrange("p (t c) -> p t c", c=C)
                with nc.allow_low_precision("fp16 sumexp; ECE tol 2e-2"):
                    if USE_CCE_FOLD:
                        nc.gpsimd.dma_start(
                            out=E3[:, :, 0:64], in_=E3[:, :, 64:128],
                            accum_op=Alu.add,
                        )
                        M = f_pool.tile([P, 64 * 64], f16, name="Mv")
                        M3 = M[:, : tr * 32].rearrange("p (t c) -> p t c", c=32)
                        nc.vector.tensor_tensor(
                            out=M3, in0=E3[:, :, 0:32], in1=E3[:, :, 32:64],
                            op=Alu.add,
                        )
                        nc.vector.tensor_reduce(
                            out=s_all[:, o : o + tr], in_=M3, axis=X, op=Alu.add
                        )
                    else:
                        cur, w = E3, C
                        while w > 16:
                            h = w // 2
                            M = f_pool.tile([P, 64 * 64], f16, name="Mv")
                            M3 = M[:, : tr * h].rearrange(
                                "p (t c) -> p t c", c=h
                            )
                            nc.vector.tensor_tensor(
                                out=M3, in0=cur[:, :, 0:h], in1=cur[:, :, h:w],
                                op=Alu.add,
                            )
                            cur, w = M3, h
                        nc.vector.tensor_reduce(
                            out=s_all[:, o : o + tr], in_=cur, axis=X, op=Alu.add
                        )

            def phase2(grp):
                c0, c1 = grp * gw, (grp + 1) * gw
                maxE = grp_pool.tile([P, gw], f32)
                nc.scalar.activation(out=maxE[:], in_=my_all[:, c0:c1],
                                     func=Act.Exp)
                rs = grp_pool.tile([P, gw], f32)
                nc.vector.reciprocal_approx_fast(out=rs[:], in_=s_all[:, c0:c1])
                nc.vector.tensor_tensor(
                    out=rhs3[:, 1, c0:c1], in0=maxE[:], in1=rs[:], op=Alu.mult
                )

                # acc: target logit attains the row max (exact in fp16)
                nc.vector.tensor_tensor(
                    out=rhs3[:, 2, c0:c1], in0=my_all[:, c0:c1],
                    in1=tl_all[:, c0:c1], op=Alu.is_equal,
                )

                # cumulative bin SIGN masks g'[k] = sign(conf - k/10), as
                # +-1 so one PSUM accumulator works across both engines.
                # Middle groups run on the scalar engine (it has slack);
                # head/tail groups stay on the DVE (is_gt then a cheap
                # single-src 2x fixup g -> 2g-1).
                on_act = grp in (0, 1, 2)
                if on_act:
                    g = grp_pool.tile([P, NB + 1, gw], f16, name="ga")
                    for k in range(NB + 1):
                        nc.scalar.activation(
                            out=g[:, k, :], in_=rhs3[:, 1, c0:c1],
                            func=Act.Sign, bias=thr_neg[:, k : k + 1],
                        )
                else:
                    g = grp_pool.tile([P, gw, NB + 1], f16, name="gv")
                    cb = rhs3[:, 1, c0:c1].unsqueeze(2).broadcast_to(
                        [P, gw, NB + 1]
                    )
                    tb = thr_sb[:].unsqueeze(1).broadcast_to([P, gw, NB + 1])
                    nc.vector.tensor_tensor(out=g[:], in0=cb, in1=tb,
                                            op=Alu.is_gt)
                    gf = g[:].rearrange("p a b -> p (a b)")
                    nc.vector.tensor_scalar(
                        out=gf, in0=gf, scalar1=2.0, scalar2=-1.0,
                        op0=Alu.mult, op1=Alu.add,
                    )

                # per-128-row-block cumulative histogram triples on PE
                for j in range(gw):
                    nc.tensor.matmul(
                        pstats[:],
                        rhs3[:, :, c0 + j],
                        g[:, :, j] if on_act else g[:, j, :],
                        start=(grp == 0 and j == 0),
                        stop=(grp == ngroups - 1 and j == gw - 1),
                        skip_group_check=True,
                    )

            pending = 0
            done_cols = 0
            for t in range(len(tiles)):
                phase1(t)
                done_cols += tiles[t][1]
                while pending < ngroups and done_cols >= (pending + 1) * gw + 16:
                    phase2(pending)
                    pending += 1
            while pending < ngroups:
                phase2(pending)
                pending += 1

            stats_sb = single.tile([3, NB + 1], f32)
            nc.vector.tensor_copy(out=stats_sb[:], in_=pstats[:])
            nc.sync.dma_start(out=out_d[:], in_=stats_sb[:])

    nc.compile()
    return nc


def prep_inputs(logits, targets, ncores=NCORES):
    """Convert + shard host inputs. Returns list of per-core in_maps."""
    l = np.asarray(logits, dtype=np.float32)
    tg = np.asarray(targets).astype(np.int64)
    n = l.shape[0]

    y16 = l.astype(np.float16)
    tl16 = y16[np.arange(n), tg]
    thr = (np.arange(NB + 1, dtype=np.float16) / NB).reshape(1, NB + 1)
    thr = thr.astype(np.float16)

    rpc = n // ncores
    in_maps = []
    for k in range(ncores):
        yk = y16[k * rpc : (k + 1) * rpc].reshape(P, TW * C)
        tlk = tl16[k * rpc : (k + 1) * rpc].reshape(P, TW)
        in_maps.append(
            {"y": np.ascontiguousarray(yk), "tl": np.ascontiguousarray(tlk),
             "thr": thr}
        )
    return in_maps


def finalize(gstats_list, n=N, sign_masks=True):
    """Combine per-core cumulative [3, 11] stats into the ECE scalar."""
    Gp = np.zeros((3, NB + 1), dtype=np.float64)
    for gs in gstats_list:
        Gp += gs.astype(np.float64)
    if sign_masks:
        # sign masks: G'[j,k] = 2*G[j,k] - S_j with S_j = G'[j,0]
        G = (Gp + Gp[:, 0:1]) / 2.0
    else:
        G = Gp
    per = G[:, 0:NB] - G[:, 1 : NB + 1]
    counts, sum_conf, sum_acc = per[0], per[1], per[2]
    safe = np.maximum(counts, 1.0)
    avg_conf = sum_conf / safe
    avg_acc = sum_acc / safe
    prop = counts / float(n)
    ece = np.where(counts > 0, np.abs(avg_conf - avg_acc) * prop, 0.0).sum()
    return np.array(ece, dtype=np.float32)


LAST_RESULTS = None  # BassKernelResults of the most recent kernel() call


def kernel(logits, targets):
    global LAST_RESULTS
    from concourse.bass_utils import run_bass_kernel_spmd

    key = (SEGMENTS, GW, tuple(sorted(KNOBS.items())))
    if key not in _CACHE:
        _CACHE[key] = build(SEGMENTS, GW, **KNOBS)
    nc = _CACHE[key]

    in_maps = prep_inputs(logits, targets)
    res = run_bass_kernel_spmd(nc, in_maps, core_ids=list(range(NCORES)))
    LAST_RESULTS = res
    return finalize([r["gstats"] for r in res.results], sign_masks=True)


# revision 50
# speedup vs baseline: 1.1826x; 1.1826x over previous
"""Trainium2 Bass kernel for expected-calibration-error (ECE) over [N,C] logits.

Contract: kernel(logits, targets) -> np.float32 scalar (shape ()), matching

    probs = softmax(logits); conf = max(probs); pred = argmax(probs)
    acc = (pred == targets); bins of width 1/10 over (k/10, (k+1)/10]
    ECE = sum_k |avg_conf_k - avg_acc_k| * count_k / N

Strategy (data-parallel over 8 NeuronCores, rows sharded):
  * Host converts logits to fp16 and gathers the target-class logit per
    row (tl). This halves HBM traffic and unlocks the DVE 2x perf mode
    (2-byte dtypes) for tensor_tensor fold chains. ECE is an aggregate
    over 262k rows with a 2e-2 tolerance; fp16's 1e-3-level noise on conf
    and ~0.2%-of-rows argmax-tie noise on acc are far below the gate.
  * Per tile [128 partitions, T rows, 128 classes], both row reductions
    run as DVE tensor_tensor fold chains 128->64->32->16 at the fp16 2x
    perf mode, then a short 1x tensor_reduce. GpSimd stays off the
    shared DVE/GpSimd SBUF port, so the TTs hold their 2x rate (a
    concurrent GpSimd fold was measured to halve DVE TT throughput).
  * acc = (tl == rowmax), exact in the fp16 domain (tl is one of the
    row's values, so equality holds iff the target attains the max).
  * conf = exp(rowmax) * reciprocal_approx(sumexp).
  * Bin masks as cumulative +-1 masks sign(conf - k/10), engine-split to
    balance load: groups 0-2 on the scalar engine (11 per-bin Sign
    activations with the negated threshold as per-partition bias; Exp
    and Sign share one act table set), the tail group on the DVE
    (is_gt broadcast, then a single-src 2x fixup g -> 2g-1 so one PSUM
    convention holds). The tensor engine accumulates cumulative [3,11]
    (count,conf,acc) sign-stats in PSUM via one tiny fp16 matmul per
    128-row block; the host recovers G = (G' + G'[:,0:1])/2, sums the
    cores' [3,11], differences adjacent columns, applies ECE.
  * The first tiles are small (head ramp): compute starts as soon as the
    first 0.5 MB lands instead of after a full 2 MB tile.

Dormant knobs kept for reference: USE_CCE_FOLD=True folds the sumexp via
in-place CCE-add DMAs. Measured verdict: works only when each accum DMA
stays <= ~4096 descriptors (32 rows x 128 partitions; bigger wedges the
SWDGE descriptor ring at runtime), and is uneconomical regardless — Q7
descriptor generation occupies gpsimd 2-3.4us per strided fold DMA,
matching the DVE TT it replaces while serializing the pipeline (126us
vs 66us measured). masks_eng='gpsimd' (gpsimd-subtract + scalar-engine
Sign masks) is untested in isolation; finalize(sign_masks=True) holds
the matching host-side math.
"""

import numpy as np

# Problem constants (hardcoded per harness contract).
N = 262144
C = 128
P = 128          # SBUF partitions
NB = 10          # calibration bins
NCORES = 8
ROWS_PER_CORE = N // NCORES          # 32768
TW = ROWS_PER_CORE // P              # 256 row-blocks (columns) per core

# tile segments: (rows-per-partition, count). Must sum to TW.
SEGMENTS = ((16, 2), (32, 1), (64, 3))
GW = 64          # phase-2 group width (columns)

_CACHE = {}

USE_CCE_FOLD = False

# masks_eng: 'gpsimd' = gpsimd subtract + ACT sign; 'dve' = DVE is_gt
KNOBS = dict(masks_eng="dve")


def build(segments=SEGMENTS, gw=GW, masks_eng="gpsimd"):
    """Build the Bass module. Returns nc."""
    import concourse.bacc as bacc
    import concourse.tile as tile
    from concourse import mybir

    f32 = mybir.dt.float32
    f16 = mybir.dt.float16
    Alu = mybir.AluOpType
    Act = mybir.ActivationFunctionType
    X = mybir.AxisListType.X

    tiles = []  # (col_offset, t_rows)
    off = 0
    for t_rows, cnt in segments:
        for _ in range(cnt):
            tiles.append((off, t_rows))
            off += t_rows
    assert off == TW
    assert TW % gw == 0
    ngroups = TW // gw

    nc = bacc.Bacc(trn_type="TRN2")

    y_d = nc.dram_tensor("y", [P, TW * C], f16, kind="ExternalInput")
    tl_d = nc.dram_tensor("tl", [P, TW], f16, kind="ExternalInput")
    thr_d = nc.dram_tensor("thr", [1, NB + 1], f16, kind="ExternalInput")
    out_d = nc.dram_tensor("gstats", [3, NB + 1], f32, kind="ExternalOutput")

    with tile.TileContext(nc) as tc:
        with (
            tc.tile_pool(name="io", bufs=4) as io_pool,
            tc.tile_pool(name="ep", bufs=3) as e_pool,
            tc.tile_pool(name="fp", bufs=3) as f_pool,
            tc.tile_pool(name="grp", bufs=3) as grp_pool,
            tc.tile_pool(name="single", bufs=1) as single,
            tc.tile_pool(name="psum", bufs=1, space="PSUM") as psum_pool,
        ):
            # Pre-issue the first y-tile DMAs (head ramp).
            y_tiles = {}
            for t in range(min(4, len(tiles))):
                o, tr = tiles[t]
                y_t = io_pool.tile([P, 64 * C], f16, name="y_t")
                nc.sync.dma_start(
                    out=y_t[:, : tr * C], in_=y_d[:, o * C : (o + tr) * C]
                )
                y_tiles[t] = y_t

            tl_all = single.tile([P, TW], f16)
            nc.sync.dma_start(out=tl_all[:], in_=tl_d[:])
            thr_sb = single.tile([P, NB + 1], f16)
            nc.sync.dma_start(out=thr_sb[:], in_=thr_d[:].partition_broadcast(P))
            # negated thresholds (fp32) as per-partition bias for ACT Sign
            thr_neg = single.tile([P, NB + 1], f32)
            nc.vector.tensor_scalar(
                out=thr_neg[:], in0=thr_sb[:], scalar1=-1.0, scalar2=None,
                op0=Alu.mult,
            )

            pstats = psum_pool.tile([3, NB + 1], f32)
            my_all = single.tile([P, TW], f16)
            s_all = single.tile([P, TW], f32)
            rhs3 = single.tile([P, 3, TW], f16)
            nc.gpsimd.memset(rhs3[:, 0, :], 1.0)

            def max_chain(src3, t_rows, out_col):
                """DVE TT max fold chain 128->64->32->16, then reduce."""
                cur, w = src3, C
                while w > 16:
                    h = w // 2
                    M = f_pool.tile([P, 64 * 64], f16, name="Mv")
                    M3 = M[:, : t_rows * h].rearrange("p (t c) -> p t c", c=h)
                    nc.vector.tensor_tensor(
                        out=M3, in0=cur[:, :, 0:h], in1=cur[:, :, h:w], op=Alu.max
                    )
                    cur, w = M3, h
                nc.vector.tensor_reduce(out=out_col, in_=cur, axis=X, op=Alu.max)

            def phase1(t):
                o, tr = tiles[t]
                y_t = y_tiles.pop(t, None)
                if y_t is None:
                    y_t = io_pool.tile([P, 64 * C], f16, name="y_t")
                    nc.sync.dma_start(
                        out=y_t[:, : tr * C], in_=y_d[:, o * C : (o + tr) * C]
                    )
                y3 = y_t[:, : tr * C].rearrange("p (t c) -> p t c", c=C)

                # row max: DVE fold chain (2x TT) + short reduce
                max_chain(y3, tr, my_all[:, o : o + tr])

                # row sumexp: fp16 exp, then a DVE add fold chain + reduce
                E = e_pool.tile([P, 64 * C], f16, name="E")
                nc.scalar.activation(out=E[:, : tr * C], in_=y_t[:, : tr * C],
                                     func=Act.Exp)
                E3 = E[:, : tr * C].rearrange("p (t c) -> p t c", c=C)
                with nc.allow_low_precision("fp16 sumexp; ECE tol 2e-2"):
                    if USE_CCE_FOLD:
                        nc.gpsimd.dma_start(
                            out=E3[:, :, 0:64], in_=E3[:, :, 64:128],
                            accum_op=Alu.add,
                        )
                        M = f_pool.tile([P, 64 * 64], f16, name="Mv")
                        M3 = M[:, : tr * 32].rearrange("p (t c) -> p t c", c=32)
                        nc.vector.tensor_tensor(
                            out=M3, in0=E3[:, :, 0:32], in1=E3[:, :, 32:64],
                            op=Alu.add,
                        )
                        nc.vector.tensor_reduce(
                            out=s_all[:, o : o + tr], in_=M3, axis=X, op=Alu.add
                        )
                    else:
                        cur, w = E3, C
                        while w > 16:
                            h = w // 2
                            M = f_pool.tile([P, 64 * 64], f16, name="Mv")
                            M3 = M[:, : tr * h].rearrange(
                                "p (t c) -> p t c", c=h
                            )
                            nc.vector.tensor_tensor(
                                out=M3, in0=cur[:, :, 0:h], in1=cur[:, :, h:w],
                                op=Alu.add,
                            )
                            cur, w = M3, h
                        nc.vector.tensor_reduce(
                            out=s_all[:, o : o + tr], in_=cur, axis=X, op=Alu.add
                        )

            def phase2(grp):
                c0, c1 = grp * gw, (grp + 1) * gw
                maxE = grp_pool.tile([P, gw], f32)
                nc.scalar.activation(out=maxE[:], in_=my_all[:, c0:c1],
                                     func=Act.Exp)
                rs = grp_pool.tile([P, gw], f32)
                nc.vector.reciprocal_approx_fast(out=rs[:], in_=s_all[:, c0:c1])
                nc.vector.tensor_tensor(
                    out=rhs3[:, 1, c0:c1], in0=maxE[:], in1=rs[:], op=Alu.mult
                )

                # acc: target logit attains the row max (exact in fp16)
                nc.vector.tensor_tensor(
                    out=rhs3[:, 2, c0:c1], in0=my_all[:, c0:c1],
                    in1=tl_all[:, c0:c1], op=Alu.is_equal,
                )

                # cumulative bin SIGN masks g'[k] = sign(conf - k/10), as
                # +-1 so one PSUM accumulator works across both engines.
                # Middle groups run on the scalar engine (it has slack);
                # head/tail groups stay on the DVE (is_gt then a cheap
                # single-src 2x fixup g -> 2g-1).
                on_act = grp in (0, 1, 2)
                if on_act:
                    g = grp_pool.tile([P, NB + 1, gw], f16, name="ga")
                    for k in range(NB + 1):
                        nc.scalar.activation(
                            out=g[:, k, :], in_=rhs3[:, 1, c0:c1],
                            func=Act.Sign, bias=thr_neg[:, k : k + 1],
                        )
                else:
                    g = grp_pool.tile([P, gw, NB + 1], f16, name="gv")
                    cb = rhs3[:, 1, c0:c1].unsqueeze(2).broadcast_to(
                        [P, gw, NB + 1]
                    )
                    tb = thr_sb[:].unsqueeze(1).broadcast_to([P, gw, NB + 1])
                    nc.vector.tensor_tensor(out=g[:], in0=cb, in1=tb,
                                            op=Alu.is_gt)
                    gf = g[:].rearrange("p a b -> p (a b)")
                    nc.vector.tensor_scalar(
                        out=gf, in0=gf, scalar1=2.0, scalar2=-1.0,
                        op0=Alu.mult, op1=Alu.add,
                    )

                # per-128-row-block cumulative histogram triples on PE
                for j in range(gw):
                    nc.tensor.matmul(
                        pstats[:],
                        rhs3[:, :, c0 + j],
                        g[:, :, j] if on_act else g[:, j, :],
                        start=(grp == 0 and j == 0),
                        stop=(grp == ngroups - 1 and j == gw - 1),
                        skip_group_check=True,
                    )

            pending = 0
            done_cols = 0
            for t in range(len(tiles)):
                phase1(t)
                done_cols += tiles[t][1]
                while pending < ngroups and done_cols >= (pending + 1) * gw + 16:
                    phase2(pending)
                    pending += 1
            while pending < ngroups:
                phase2(pending)
                pending += 1

            stats_sb = single.tile([3, NB + 1], f32)
            nc.vector.tensor_copy(out=stats_sb[:], in_=pstats[:])
            nc.sync.dma_start(out=out_d[:], in_=stats_sb[:])

    nc.compile()
    return nc


def prep_inputs(logits, targets, ncores=NCORES):
    """Convert + shard host inputs. Returns list of per-core in_maps."""
    l = np.asarray(logits, dtype=np.float32)
    tg = np.asarray(targets).astype(np.int64)
    n = l.shape[0]

    y16 = l.astype(np.float16)
    tl16 = y16[np.arange(n), tg]
    thr = (np.arange(NB + 1, dtype=np.float16) / NB).reshape(1, NB + 1)
    thr = thr.astype(np.float16)

    rpc = n // ncores
    in_maps = []
    for k in range(ncores):
        yk = y16[k * rpc : (k + 1) * rpc].reshape(P, TW * C)
        tlk = tl16[k * rpc : (k + 1) * rpc].reshape(P, TW)
        in_maps.append(
            {"y": np.ascontiguousarray(yk), "tl": np.ascontiguousarray(tlk),
             "thr": thr}
        )
    return in_maps


def finalize(gstats_list, n=N, sign_masks=True):
    """Combine per-core cumulative [3, 11] stats into the ECE scalar."""
    Gp = np.zeros((3, NB + 1), dtype=np.float64)
    for gs in gstats_list:
        Gp += gs.astype(np.float64)
    if sign_masks:
        # sign masks: G'[j,k] = 2*G[j,k] - S_j with S_j = G'[j,0]
        G = (Gp + Gp[:, 0:1]) / 2.0
    else:
        G = Gp
    per = G[:, 0:NB] - G[:, 1 : NB + 1]
    counts, sum_conf, sum_acc = per[0], per[1], per[2]
    safe = np.maximum(counts, 1.0)
    avg_conf = sum_conf / safe
    avg_acc = sum_acc / safe
    prop = counts / float(n)
    ece = np.where(counts > 0, np.abs(avg_conf - avg_acc) * prop, 0.0).sum()
    return np.array(ece, dtype=np.float32)


LAST_RESULTS = None  # BassKernelResults of the most recent kernel() call


def kernel(logits, targets):
    global LAST_RESULTS
    from concourse.bass_utils import run_bass_kernel_spmd

    key = (SEGMENTS, GW, tuple(sorted(KNOBS.items())))
    if key not in _CACHE:
        _CACHE[key] = build(SEGMENTS, GW, **KNOBS)
    nc = _CACHE[key]

    in_maps = prep_inputs(logits, targets)
    res = run_bass_kernel_spmd(nc, in_maps, core_ids=list(range(NCORES)))
    LAST_RESULTS = res
    return finalize([r["gstats"] for r in res.results], sign_masks=True)
